# revision 1
# baseline (speedup 1.0000x reference)
"""Trainium2 Bass kernel for nn_CrossNetwork: 4-layer cross-network.

Reference semantics (per row b of x [B, D], D=512, L=4 layers):
    x_list = [x]
    for i in range(L):
        h = x_list[-1]
        for p in x_list[:-1]:          # sequential dot-product residuals
            s = <h_cur, p>             # scalar per row (h_cur updated each step)
            h_cur = h_cur + s * ones
        y = h_cur @ W[i].T + b[i]
        x_list.append(y)
    out = concat(x_list[1:])           # [B, L*D]

Key algebraic restructure (exact): adding a per-row scalar s to every
component only shifts later dot products by s * rowsum(prior).  With
D_j = <h, p_j> (h = the layer input, unmodified) and sig_j = rowsum(p_j):
    s'_j = D_j + S_{<j} * sig_j ;  S = sum_j s'_j
so only the plain dots D_j, the row-sums sig_j of y0/y1, and a tiny
per-row recurrence are needed; the shift S is applied once per layer.

Layout: batch rows on SBUF partitions ([128, 512] tiles), activations f32.
Matmul stationary = PE-transposed activation chunks; moving = host-
pre-transposed W^T.  Bias via an extra K=1 accumulating matmul.
Sharding: batch split across 8 NeuronCores (data parallel, SPMD).
"""

import numpy as np

NUM_LAYERS = 4
D = 512
B = 16384
N_CORES = 8
ROWS_PER_CORE = B // N_CORES          # 2048
NTILES = ROWS_PER_CORE // 128         # 16
NCH = D // 128                        # 4 contraction chunks

# matmul operand dtype: "bf16" or "f32r"
MM_DTYPE = "f32r"
# row-dot reduction: "ts_accum" (mul + tensor_scalar reduce) or
# "reduce" (mul + tensor_reduce)
DOT_MODE = "ts_accum"

_CACHE = {}


def _build_nc(ntiles=NTILES):
    import concourse.tile as tile
    from concourse import bacc, mybir
    from concourse.masks import make_identity

    F32 = mybir.dt.float32
    BF16 = mybir.dt.bfloat16
    F32R = mybir.dt.float32r
    AF = mybir.ActivationFunctionType
    MUL = mybir.AluOpType.mult
    ADD = mybir.AluOpType.add

    MMDT = F32R if MM_DTYPE == "f32r" else BF16
    FINDT = F32 if MM_DTYPE == "f32r" else BF16
    rows = ntiles * 128

    nc = bacc.Bacc("TRN2", target_bir_lowering=False, debug=False)

    X = nc.dram_tensor("x", [rows, D], F32, kind="ExternalInput")
    WT = nc.dram_tensor("wt", [NUM_LAYERS, D, D], MMDT, kind="ExternalInput")
    BIAS = nc.dram_tensor("bias", [NUM_LAYERS, D], MMDT, kind="ExternalInput")
    OUT = nc.dram_tensor("out", [rows, NUM_LAYERS * D], F32,
                         kind="ExternalOutput")

    with tile.TileContext(nc) as tc:
        with (
            tc.tile_pool(name="consts", bufs=1) as consts,
            tc.tile_pool(name="acts", bufs=2) as acts,
            tc.tile_pool(name="fins", bufs=3) as fins,
            tc.tile_pool(name="scratch", bufs=2) as scratch,
            tc.tile_pool(name="scals", bufs=2) as scals,
            tc.tile_pool(name="ypsum", bufs=3, space="PSUM") as ypsum,
            tc.tile_pool(name="tpsum", bufs=3, space="PSUM") as tpsum,
        ):
            # ---- constants (loaded once) ----
            wt_sb = consts.tile([128, NUM_LAYERS, NCH, D], MMDT)
            wt_dram = WT.rearrange("l (c p) e -> l c p e", p=128)
            for i in range(NUM_LAYERS):
                for c in range(NCH):
                    nc.sync.dma_start(wt_sb[:, i, c, :], wt_dram[i, c, :, :])
            bias_sb = consts.tile([1, NUM_LAYERS, D], MMDT)
            for i in range(NUM_LAYERS):
                nc.sync.dma_start(bias_sb[0:1, i, :], BIAS[i:i + 1, :])
            ones_f32 = consts.tile([1, 128], F32)
            nc.vector.memset(ones_f32[:], 1.0)
            ones_row = consts.tile([1, 128], MMDT)
            nc.vector.tensor_copy(ones_row[:], ones_f32[:])
            ident = consts.tile([128, 128], FINDT)
            make_identity(nc, ident[:])

            x_dram = X.rearrange("(t p) d -> t p d", p=128)
            out_dram = OUT.rearrange("(t p) d -> t p d", p=128)

            def row_reduce(src_ap, dst_col, tag):
                """dst_col[128,1] = rowsum(src_ap [128,D])."""
                if DOT_MODE == "ts_accum":
                    waste = scratch.tile([128, D], F32, tag=tag)
                    nc.vector.tensor_scalar(
                        out=waste[:], in0=src_ap, scalar1=0.0, scalar2=None,
                        op0=ADD, op1=ADD, accum_out=dst_col)
                else:
                    nc.vector.tensor_reduce(
                        out=dst_col, in_=src_ap, op=ADD,
                        axis=mybir.AxisListType.X)

            for t in range(ntiles):
                # ---- load x tile ----
                x_t = acts.tile([128, D], F32, tag="x")
                nc.sync.dma_start(x_t[:], x_dram[t, :, :])

                ys = []      # f32 activation tiles [x_t, y0, y1, y2]
                sigs = {}    # rowsum columns for y0, y1

                scal = scals.tile([128, 16], F32, tag="scal")
                ncol = [0]
                def col():
                    c = ncol[0]; ncol[0] += 1
                    return scal[:, c:c + 1]

                h = x_t
                ys.append(x_t)

                for i in range(NUM_LAYERS):
                    # ---- dots vs priors + recurrence -> S (skip layer 0) ----
                    S = None
                    if i >= 1:
                        Ds = []
                        for j, p in enumerate(ys[:-1]):
                            prod = scratch.tile([128, D], F32, tag="prod")
                            nc.vector.tensor_tensor(
                                out=prod[:], in0=h[:], in1=p[:], op=MUL)
                            Dj = col()
                            row_reduce(prod[:], Dj, "dotred")
                            Ds.append(Dj)
                        if i == 1:
                            S = Ds[0]
                        elif i == 2:
                            # S = D0 + D1 + D0*sig(y0)
                            u = col()
                            nc.vector.tensor_scalar(
                                out=u, in0=sigs[0], scalar1=Ds[0], scalar2=Ds[0],
                                op0=MUL, op1=ADD)  # u = sig0*D0 + D0
                            S = col()
                            nc.vector.tensor_scalar(
                                out=S, in0=u, scalar1=Ds[1], scalar2=None, op0=ADD)
                        else:
                            # priors x, y0, y1 with sig(y0), sig(y1)
                            u = col()
                            nc.vector.tensor_scalar(
                                out=u, in0=sigs[0], scalar1=Ds[0], scalar2=Ds[0],
                                op0=MUL, op1=ADD)          # u = D0*(1+sig0)
                            sa = col()
                            nc.vector.tensor_scalar(
                                out=sa, in0=u, scalar1=Ds[1], scalar2=None, op0=ADD)
                            v = col()
                            nc.vector.tensor_scalar(
                                out=v, in0=sigs[1], scalar1=sa, scalar2=sa,
                                op0=MUL, op1=ADD)          # v = sa*(1+sig1)
                            S = col()
                            nc.vector.tensor_scalar(
                                out=S, in0=v, scalar1=Ds[2], scalar2=None, op0=ADD)

                    # ---- x_fin = h + S (gpsimd; casts when FINDT != F32) ----
                    if S is None:
                        if FINDT == F32:
                            x_fin = h
                        else:
                            x_fin = fins.tile([128, D], FINDT, tag="fin")
                            nc.gpsimd.tensor_copy(x_fin[:], h[:])
                    else:
                        x_fin = fins.tile([128, D], FINDT, tag="fin")
                        nc.gpsimd.tensor_scalar_add(x_fin[:], h[:], S)

                    # ---- transpose x_fin -> stationary chunks ----
                    tr = tpsum.tile([128, NCH, 128], FINDT, tag="tr")
                    for c in range(NCH):
                        nc.tensor.transpose(
                            tr[:, c, :], x_fin[:, c * 128:(c + 1) * 128], ident[:])
                    xT = fins.tile([128, NCH, 128], MMDT, tag="xT")
                    nc.scalar.activation(xT[:], tr[:], AF.Copy)

                    # ---- matmuls: y = x_fin @ W_i^T + bias ----
                    y_ps = ypsum.tile([128, D], F32, tag="y")
                    for c in range(NCH):
                        nc.tensor.matmul(
                            y_ps[:], xT[:, c, :], wt_sb[:, i, c, :],
                            start=(c == 0), stop=False)
                    nc.tensor.matmul(
                        y_ps[:], ones_row[:], bias_sb[:, i, :],
                        start=False, stop=True)

                    # ---- P1: copy y psum -> sbuf f32; sigma for y0, y1 ----
                    y = acts.tile([128, D], F32, tag=f"y{i}")
                    nc.scalar.activation(y[:], y_ps[:], AF.Copy)
                    if i in (0, 1):
                        sig = col()
                        row_reduce(y[:], sig, "sigred")
                        sigs[i] = sig

                    # ---- DMA out ----
                    nc.sync.dma_start(out_dram[t, :, i * D:(i + 1) * D], y[:])

                    ys.append(y)
                    h = y

    nc.compile()
    return nc


def _host_prep(W, b):
    """W [L,D,D] f32 (torch Linear layout: y = x @ W.T) -> transposed WT[l,d,e]."""
    WT = np.ascontiguousarray(W.transpose(0, 2, 1))
    bias = np.ascontiguousarray(b)
    if MM_DTYPE == "f32r":
        # PE accepts raw f32 bits for f32r DRAM operands (verified on HW:
        # identical error to DVE-rounded) — no host rounding needed.
        return WT, bias
    else:
        import ml_dtypes
        return (np.asarray(WT, dtype=ml_dtypes.bfloat16),
                np.asarray(bias, dtype=ml_dtypes.bfloat16))


def run_shards(x, W, b, **spmd_kwargs):
    """Run the SPMD kernel; returns (full_output, BassKernelResults)."""
    from concourse.bass_utils import run_bass_kernel_spmd

    x = np.ascontiguousarray(np.asarray(x, np.float32))
    WT, bias = _host_prep(np.asarray(W, np.float32), np.asarray(b, np.float32))

    if "nc" not in _CACHE:
        _CACHE["nc"] = _build_nc()
    nc = _CACHE["nc"]

    in_maps = []
    for c in range(N_CORES):
        shard = x[c * ROWS_PER_CORE:(c + 1) * ROWS_PER_CORE]
        in_maps.append({"x": np.ascontiguousarray(shard), "wt": WT, "bias": bias})

    res = run_bass_kernel_spmd(nc, in_maps, core_ids=list(range(N_CORES)),
                               **spmd_kwargs)
    out = np.concatenate([r["out"] for r in res.results], axis=0)
    return out.astype(np.float32), res


def kernel(x, W, b):
    out, _ = run_shards(x, W, b)
    return out



# revision 3
# speedup vs baseline: 4.9773x; 4.9773x over previous
"""Trainium2 Bass kernel for nn_CrossNetwork: 4-layer cross-network.

Reference semantics (per row b of x [B, D], D=512, L=4 layers):
    x_list = [x]
    for i in range(L):
        h = x_list[-1]
        for p in x_list[:-1]:          # sequential dot-product residuals
            s = <h_cur, p>             # scalar per row (h_cur updated each step)
            h_cur = h_cur + s * ones
        y = h_cur @ W[i].T + b[i]
        x_list.append(y)
    out = concat(x_list[1:])           # [B, L*D]

Algebraic restructure (exact): with D_j = <h, p_j> (h = raw layer input)
and sig_j = rowsum(p_j), the accumulated shift S satisfies
    s'_j = D_j + S_{<j} * sig_j ;  S = sum_j s'_j
and since y = (h + S*1) @ W^T + b = h @ W^T + S * wbar + b  (wbar = W.sum(-1)),
the shift never needs to be materialized: it enters as a rank-1 PSUM update.

Dataflow: TRANSPOSED activations. x is host-transposed to x^T [D, B]; all
layers compute y^T[e, b] = sum_d W[e, d] h^T[d, b] with the weight chunk
stationary ([d,e] = W^T chunk) and activations moving -- no PE transposes,
no activation copies for stationarization. Dots <h, p> become elementwise
products (DVE) + partition-dim reduction (ones-stationary matmul). The
bias is folded into the PSUM->SBUF copy (ACT Identity with per-partition
bias). Everything f32r (TF32-like matmul dtype, raw f32 bits in DRAM).

Sharding: batch split across 8 NeuronCores (data parallel, SPMD).
Output written transposed ([L*D, B] per core); host re-transposes.
"""

import numpy as np

NUM_LAYERS = 4
D = 512
B = 16384
N_CORES = 8
COLS_PER_CORE = B // N_CORES          # 2048 batch columns per core
NB = 512                              # batch columns per tile (moving N)
NBT = COLS_PER_CORE // NB             # 4 batch tiles
NCH = D // 128                        # 4 feature chunks (d and e)

_CACHE = {}


def _build_nc():
    import concourse.tile as tile
    from concourse import bacc, mybir

    F32 = mybir.dt.float32
    F32R = mybir.dt.float32r
    AF = mybir.ActivationFunctionType
    MUL = mybir.AluOpType.mult
    ADD = mybir.AluOpType.add

    nc = bacc.Bacc("TRN2", target_bir_lowering=False, debug=False)

    XT = nc.dram_tensor("xt", [D, COLS_PER_CORE], F32R, kind="ExternalInput")
    WT = nc.dram_tensor("wt", [NUM_LAYERS, D, D], F32R, kind="ExternalInput")
    WBAR = nc.dram_tensor("wbar", [NUM_LAYERS, D], F32R, kind="ExternalInput")
    BIASC = nc.dram_tensor("biasc", [128, NUM_LAYERS * NCH], F32,
                           kind="ExternalInput")
    OUT = nc.dram_tensor("out", [NUM_LAYERS * D, COLS_PER_CORE], F32R,
                         kind="ExternalOutput")

    out_r = OUT.rearrange("(l e p) b -> l e p b", e=NCH, p=128)
    xt_dram = XT.rearrange("(c p) b -> c p b", p=128)
    wt_dram = WT.rearrange("l (c p) e -> l c p e", p=128)

    with tile.TileContext(nc) as tc:
        with (
            tc.tile_pool(name="consts", bufs=1) as consts,
            tc.tile_pool(name="acts", bufs=3) as acts,
            tc.tile_pool(name="prods", bufs=4) as prods,
            tc.tile_pool(name="rows", bufs=2) as rows,
            tc.tile_pool(name="ypsum", bufs=4, space="PSUM") as ypsum,
            tc.tile_pool(name="sigpsum", bufs=1, space="PSUM") as sigpsum,
            tc.tile_pool(name="dotpsum", bufs=3, space="PSUM") as dotpsum,
        ):
            # ---- constants (loaded once) ----
            wt_sb = consts.tile([128, NUM_LAYERS, NCH, D], F32R)
            for i in range(NUM_LAYERS):
                for c in range(NCH):
                    nc.sync.dma_start(wt_sb[:, i, c, :], wt_dram[i, c, :, :])
            wbar_sb = consts.tile([1, NUM_LAYERS, D], F32R)
            for i in range(NUM_LAYERS):
                nc.sync.dma_start(wbar_sb[0:1, i, :], WBAR[i:i + 1, :])
            biasc = consts.tile([128, NUM_LAYERS * NCH], F32)
            nc.sync.dma_start(biasc[:], BIASC[:, :])
            ones_f = consts.tile([128, 32], F32)
            nc.vector.memset(ones_f[:], 1.0)
            ones32 = consts.tile([128, 32], F32R)
            nc.vector.tensor_copy(ones32[:], ones_f[:])

            xt_sb = consts.tile([128, NCH, COLS_PER_CORE], F32R)
            for c in range(NCH):
                nc.sync.dma_start(xt_sb[:, c, :], xt_dram[c, :, :])

            for bt in range(NBT):
                cs = slice(bt * NB, (bt + 1) * NB)

                def x_chunk(c, cs=cs):
                    return xt_sb[:, c, cs]

                ys = []          # y tiles [128, NCH, NB] per layer
                sps = {}         # sig-plus-one rows [1, NB]
                S = None         # current shift row [1, NB] (f32r sbuf)

                for i in range(NUM_LAYERS):
                    h_chunk = x_chunk if i == 0 else \
                        (lambda c, t=ys[i - 1]: t[:, c, :])

                    # ---- main matmuls + rank-1 shift into PSUM ----
                    y_t = acts.tile([128, NCH, NB], F32R, tag=f"y{i}")
                    for e in range(NCH):
                        yp = ypsum.tile([128, NB], F32, tag="y")
                        for c in range(NCH):
                            nc.tensor.matmul(
                                yp[:], wt_sb[:, i, c, e * 128:(e + 1) * 128],
                                h_chunk(c), start=(c == 0),
                                stop=(c == 3 and i == 0))
                        if i >= 1:
                            nc.tensor.matmul(
                                yp[:], wbar_sb[0:1, i, e * 128:(e + 1) * 128],
                                S[0:1, :], start=False, stop=True)
                        # ---- copy to SBUF with bias fold; DMA out ----
                        nc.scalar.activation(
                            y_t[:, e, :], yp[:], AF.Identity,
                            bias=biasc[:, i * NCH + e:i * NCH + e + 1])
                        nc.sync.dma_start(out_r[i, e, :, cs], y_t[:, e, :])

                    # ---- rowsum(y_i) for i in (0,1): sig rows ----
                    if i in (0, 1):
                        sp_ps = sigpsum.tile([32, NB], F32, tag="sig")
                        for c in range(NCH):
                            nc.tensor.matmul(sp_ps[:], ones32[:], y_t[:, c, :],
                                             start=(c == 0), stop=(c == 3))
                        sp = rows.tile([1, NB], F32R, tag=f"sp{i}")
                        nc.scalar.activation(sp[0:1, :], sp_ps[0:1, :],
                                             AF.Copy, bias=1.0)
                        sps[i] = sp

                    # ---- dots of y_i vs priors -> next layer's S ----
                    if i < NUM_LAYERS - 1:
                        priors = [x_chunk] + [
                            (lambda c, t=ys[j]: t[:, c, :]) for j in range(i)]
                        dps = []
                        for p_chunk in priors:
                            dp = dotpsum.tile([32, NB], F32, tag="dot")
                            for c in range(NCH):
                                prod = prods.tile([128, NB], F32R, tag="prod")
                                nc.vector.tensor_tensor(
                                    out=prod[:], in0=y_t[:, c, :],
                                    in1=p_chunk(c), op=MUL)
                                nc.tensor.matmul(dp[:], ones32[:], prod[:],
                                                 start=(c == 0), stop=(c == 3))
                            dps.append(dp)

                        S = rows.tile([1, NB], F32R, tag="S")
                        if i == 0:
                            # S1 = <y0, x>
                            nc.vector.tensor_copy(S[0:1, :], dps[0][0:1, :])
                        elif i == 1:
                            # S2 = D_x*(1+sig0) + D_y0
                            t = rows.tile([1, NB], F32R, tag="t")
                            nc.vector.tensor_tensor(
                                out=t[0:1, :], in0=dps[0][0:1, :],
                                in1=sps[0][0:1, :], op=MUL)
                            nc.vector.tensor_tensor(
                                out=S[0:1, :], in0=dps[1][0:1, :],
                                in1=t[0:1, :], op=ADD)
                        else:
                            # S3 = (D_x*(1+sig0) + D_y0)*(1+sig1) + D_y1
                            t1 = rows.tile([1, NB], F32R, tag="t")
                            nc.vector.tensor_tensor(
                                out=t1[0:1, :], in0=dps[0][0:1, :],
                                in1=sps[0][0:1, :], op=MUL)
                            t2 = rows.tile([1, NB], F32R, tag="t")
                            nc.vector.tensor_tensor(
                                out=t2[0:1, :], in0=dps[1][0:1, :],
                                in1=t1[0:1, :], op=ADD)
                            t3 = rows.tile([1, NB], F32R, tag="t")
                            nc.vector.tensor_tensor(
                                out=t3[0:1, :], in0=t2[0:1, :],
                                in1=sps[1][0:1, :], op=MUL)
                            nc.vector.tensor_tensor(
                                out=S[0:1, :], in0=dps[2][0:1, :],
                                in1=t3[0:1, :], op=ADD)

                    ys.append(y_t)

    nc.compile()
    return nc


def _host_prep(x, W, b):
    xT = np.ascontiguousarray(np.asarray(x, np.float32).T)          # [D, B]
    WT = np.ascontiguousarray(W.transpose(0, 2, 1))                 # [L, d, e]
    wbar = np.ascontiguousarray(W.sum(-1, dtype=np.float32))        # [L, D]
    # bias columns: biasc[p, i*NCH+e] = b[i, e*128+p]
    biasc = np.ascontiguousarray(
        b.reshape(NUM_LAYERS, NCH, 128).transpose(2, 0, 1)
        .reshape(128, NUM_LAYERS * NCH))
    return xT, WT, wbar, biasc


def run_shards(x, W, b, **spmd_kwargs):
    """Run the SPMD kernel; returns (full_output, BassKernelResults)."""
    from concourse.bass_utils import run_bass_kernel_spmd

    x = np.asarray(x, np.float32)
    W = np.asarray(W, np.float32)
    b = np.asarray(b, np.float32)
    xT, WT, wbar, biasc = _host_prep(x, W, b)

    if "nc" not in _CACHE:
        _CACHE["nc"] = _build_nc()
    nc = _CACHE["nc"]

    in_maps = []
    for c in range(N_CORES):
        shard = np.ascontiguousarray(
            xT[:, c * COLS_PER_CORE:(c + 1) * COLS_PER_CORE])
        in_maps.append({"xt": shard, "wt": WT, "wbar": wbar, "biasc": biasc})

    res = run_bass_kernel_spmd(nc, in_maps, core_ids=list(range(N_CORES)),
                               **spmd_kwargs)
    # per-core out: [L*D, COLS_PER_CORE] transposed; gather + re-transpose
    outT = np.concatenate(
        [np.asarray(r["out"], np.float32) for r in res.results], axis=1)
    out = np.ascontiguousarray(outT.T)                              # [B, L*D]
    return out, res


def kernel(x, W, b):
    out, _ = run_shards(x, W, b)
    return out


# revision 5
# speedup vs baseline: 5.8243x; 1.1702x over previous
"""Trainium2 Bass kernel for nn_CrossNetwork: 4-layer cross-network.

Reference semantics (per row b of x [B, D], D=512, L=4 layers):
    x_list = [x]
    for i in range(L):
        h = x_list[-1]
        for p in x_list[:-1]:          # sequential dot-product residuals
            s = <h_cur, p>             # scalar per row (h_cur updated each step)
            h_cur = h_cur + s * ones
        y = h_cur @ W[i].T + b[i]
        x_list.append(y)
    out = concat(x_list[1:])           # [B, L*D]

Algebraic restructure (exact): with D_j = <h, p_j> (h = raw layer input)
and sig_j = rowsum(p_j), the accumulated shift S satisfies
    s'_j = D_j + S_{<j} * sig_j ;  S = sum_j s'_j
and since y = (h + S*1) @ W^T + b = h @ W^T + S * wbar + b  (wbar = W.sum(-1)),
the shift never needs to be materialized: it enters as a rank-1 PSUM update.

Dataflow: TRANSPOSED activations. x is host-transposed to x^T [D, B]; all
layers compute y^T[e, b] = sum_d W[e, d] h^T[d, b] with the weight chunk
stationary ([d,e] = W^T chunk) and activations moving -- no PE transposes,
no activation copies for stationarization. Dots <h, p> become elementwise
products (DVE) + partition-dim reduction (ones-stationary matmul). The
bias is folded into the PSUM->SBUF copy (ACT Identity with per-partition
bias). Everything f32r (TF32-like matmul dtype, raw f32 bits in DRAM).

Emission is a diagonal wavefront over (batch-tile, layer) so the in-order
PE queue always has another tile's main matmuls between one tile's
S-dependent instructions.

Sharding: batch split across 8 NeuronCores (data parallel, SPMD).
Output written transposed ([L*D, B] per core); host re-transposes.
"""

import numpy as np

NUM_LAYERS = 4
D = 512
B = 16384
N_CORES = 8
COLS_PER_CORE = B // N_CORES          # 2048 batch columns per core
NB = 512                              # batch columns per tile (moving N)
NBT = COLS_PER_CORE // NB             # 4 batch tiles
NCH = D // 128                        # 4 feature chunks (d and e)

_CACHE = {}


def _build_nc():
    import concourse.tile as tile
    from concourse import bacc, mybir

    F32 = mybir.dt.float32
    F32R = mybir.dt.float32r
    AF = mybir.ActivationFunctionType
    MUL = mybir.AluOpType.mult
    ADD = mybir.AluOpType.add

    nc = bacc.Bacc("TRN2", target_bir_lowering=False, debug=False)

    XT = nc.dram_tensor("xt", [D, COLS_PER_CORE], F32R, kind="ExternalInput")
    WT = nc.dram_tensor("wt", [NUM_LAYERS, D, D], F32R, kind="ExternalInput")
    WBAR = nc.dram_tensor("wbar", [NUM_LAYERS, D], F32R, kind="ExternalInput")
    BIASC = nc.dram_tensor("biasc", [128, NUM_LAYERS * NCH], F32,
                           kind="ExternalInput")
    OUT = nc.dram_tensor("out", [NUM_LAYERS * D, COLS_PER_CORE], F32R,
                         kind="ExternalOutput")

    out_r = OUT.rearrange("(l e p) b -> l e p b", e=NCH, p=128)
    xt_dram = XT.rearrange("(c p) b -> c p b", p=128)
    wt_dram = WT.rearrange("l (c p) e -> l c p e", p=128)

    with tile.TileContext(nc) as tc:
        with (
            tc.tile_pool(name="consts", bufs=1) as consts,
            tc.tile_pool(name="acts", bufs=3) as acts,
            tc.tile_pool(name="prods", bufs=4) as prods,
            tc.tile_pool(name="rows", bufs=2) as rows,
            tc.tile_pool(name="ypsum", bufs=5, space="PSUM") as ypsum,
            tc.tile_pool(name="sigpsum", bufs=1, space="PSUM") as sigpsum,
            tc.tile_pool(name="dotpsum", bufs=2, space="PSUM") as dotpsum,
        ):
            # ---- constants; DMAs ordered by first use in the wavefront ----
            biasc = consts.tile([128, NUM_LAYERS * NCH], F32)
            nc.sync.dma_start(biasc[:], BIASC[:, :])
            wbar_sb = consts.tile([1, NUM_LAYERS, D], F32R)
            for i in range(NUM_LAYERS):
                nc.sync.dma_start(wbar_sb[0:1, i, :], WBAR[i:i + 1, :])
            ones_f = consts.tile([128, 32], F32)
            nc.vector.memset(ones_f[:], 1.0)
            ones32 = consts.tile([128, 32], F32R)
            nc.vector.tensor_copy(ones32[:], ones_f[:])

            xt_sb = consts.tile([128, NCH, COLS_PER_CORE], F32R)
            wt_sb = consts.tile([128, NUM_LAYERS, NCH, D], F32R)

            def load_xt(bt):
                cs = slice(bt * NB, (bt + 1) * NB)
                for c in range(NCH):
                    nc.sync.dma_start(xt_sb[:, c, cs], xt_dram[c, :, cs])

            def load_wt(i):
                for c in range(NCH):
                    nc.sync.dma_start(wt_sb[:, i, c, :], wt_dram[i, c, :, :])

            load_xt(0)
            load_wt(0)
            load_wt(1)
            load_xt(1)
            load_wt(2)
            load_xt(2)
            load_wt(3)
            load_xt(3)

            # ---- per-bt state ----
            class St:
                pass

            sts = []
            for bt in range(NBT):
                st = St()
                st.cs = slice(bt * NB, (bt + 1) * NB)
                st.ys = []
                st.sps = {}
                st.S = None
                sts.append(st)

            def x_chunk_of(st):
                return lambda c: xt_sb[:, c, st.cs]

            def emit_step(bt, i):
                st = sts[bt]
                x_chunk = x_chunk_of(st)
                h_chunk = x_chunk if i == 0 else \
                    (lambda c, t=st.ys[i - 1]: t[:, c, :])

                # ---- main matmuls + rank-1 shift into PSUM ----
                y_t = acts.tile([128, NCH, NB], F32R, tag=f"y{i}")
                for e in range(NCH):
                    yp = ypsum.tile([128, NB], F32, tag="y")
                    for c in range(NCH):
                        nc.tensor.matmul(
                            yp[:], wt_sb[:, i, c, e * 128:(e + 1) * 128],
                            h_chunk(c), start=(c == 0),
                            stop=(c == 3 and i == 0))
                    if i >= 1:
                        nc.tensor.matmul(
                            yp[:], wbar_sb[0:1, i, e * 128:(e + 1) * 128],
                            st.S[0:1, :], start=False, stop=True)
                    # ---- copy to SBUF with bias fold; DMA out ----
                    nc.scalar.activation(
                        y_t[:, e, :], yp[:], AF.Identity,
                        bias=biasc[:, i * NCH + e:i * NCH + e + 1])
                    nc.sync.dma_start(out_r[i, e, :, st.cs], y_t[:, e, :])

                # ---- rowsum(y_i) for i in (0,1): sig rows (needed i+2) ----
                if i in (0, 1):
                    sp_ps = sigpsum.tile([32, NB], F32, tag="sig")
                    for c in range(NCH):
                        nc.tensor.matmul(sp_ps[:], ones32[:], y_t[:, c, :],
                                         start=(c == 0), stop=(c == 3))
                    sp = rows.tile([1, NB], F32R, tag=f"sp{i}")
                    nc.scalar.activation(sp[0:1, :], sp_ps[0:1, :],
                                         AF.Copy, bias=1.0)
                    st.sps[i] = sp

                # ---- dots of y_i vs priors -> next layer's S ----
                if i < NUM_LAYERS - 1:
                    def dot(p_chunk):
                        dp = dotpsum.tile([32, NB], F32, tag="dot")
                        for c in range(NCH):
                            prod = prods.tile([128, NB], F32R, tag="prod")
                            nc.vector.tensor_tensor(
                                out=prod[:], in0=y_t[:, c, :],
                                in1=p_chunk(c), op=MUL)
                            nc.tensor.matmul(dp[:], ones32[:], prod[:],
                                             start=(c == 0), stop=(c == 3))
                        return dp

                    S = rows.tile([1, NB], F32R, tag="S")
                    if i == 0:
                        dp0 = dot(x_chunk)                  # <y0, x>
                        nc.vector.tensor_copy(S[0:1, :], dp0[0:1, :])
                    elif i == 1:
                        dp0 = dot(x_chunk)                  # <y1, x>
                        dp1 = dot(lambda c: st.ys[0][:, c, :])
                        # S2 = D_x*(1+sig0) + D_y0
                        t = rows.tile([1, NB], F32R, tag="t")
                        nc.vector.tensor_tensor(
                            out=t[0:1, :], in0=dp0[0:1, :],
                            in1=st.sps[0][0:1, :], op=MUL)
                        nc.vector.tensor_tensor(
                            out=S[0:1, :], in0=dp1[0:1, :],
                            in1=t[0:1, :], op=ADD)
                    else:
                        # S3 = (D_x*(1+sig0) + D_y0)*(1+sig1) + D_y1
                        dp0 = dot(x_chunk)                  # <y2, x>
                        dp1 = dot(lambda c: st.ys[0][:, c, :])
                        t1 = rows.tile([1, NB], F32R, tag="t")
                        nc.vector.tensor_tensor(
                            out=t1[0:1, :], in0=dp0[0:1, :],
                            in1=st.sps[0][0:1, :], op=MUL)
                        t2 = rows.tile([1, NB], F32R, tag="t")
                        nc.vector.tensor_tensor(
                            out=t2[0:1, :], in0=dp1[0:1, :],
                            in1=t1[0:1, :], op=ADD)
                        dp2 = dot(lambda c: st.ys[1][:, c, :])
                        t3 = rows.tile([1, NB], F32R, tag="t")
                        nc.vector.tensor_tensor(
                            out=t3[0:1, :], in0=t2[0:1, :],
                            in1=st.sps[1][0:1, :], op=MUL)
                        nc.vector.tensor_tensor(
                            out=S[0:1, :], in0=dp2[0:1, :],
                            in1=t3[0:1, :], op=ADD)
                    st.S = S

                st.ys.append(y_t)

            # ---- diagonal wavefront over (bt, layer), deepest layer first ----
            for diag in range(NBT + NUM_LAYERS - 1):
                for bt in range(min(diag, NBT - 1) + 1):
                    i = diag - bt
                    if 0 <= i < NUM_LAYERS:
                        emit_step(bt, i)

    nc.compile()
    return nc


def _host_prep(x, W, b):
    xT = np.ascontiguousarray(np.asarray(x, np.float32).T)          # [D, B]
    WT = np.ascontiguousarray(W.transpose(0, 2, 1))                 # [L, d, e]
    wbar = np.ascontiguousarray(W.sum(-1, dtype=np.float32))        # [L, D]
    # bias columns: biasc[p, i*NCH+e] = b[i, e*128+p]
    biasc = np.ascontiguousarray(
        b.reshape(NUM_LAYERS, NCH, 128).transpose(2, 0, 1)
        .reshape(128, NUM_LAYERS * NCH))
    return xT, WT, wbar, biasc


def run_shards(x, W, b, **spmd_kwargs):
    """Run the SPMD kernel; returns (full_output, BassKernelResults)."""
    from concourse.bass_utils import run_bass_kernel_spmd

    x = np.asarray(x, np.float32)
    W = np.asarray(W, np.float32)
    b = np.asarray(b, np.float32)
    xT, WT, wbar, biasc = _host_prep(x, W, b)

    if "nc" not in _CACHE:
        _CACHE["nc"] = _build_nc()
    nc = _CACHE["nc"]

    in_maps = []
    for c in range(N_CORES):
        shard = np.ascontiguousarray(
            xT[:, c * COLS_PER_CORE:(c + 1) * COLS_PER_CORE])
        in_maps.append({"xt": shard, "wt": WT, "wbar": wbar, "biasc": biasc})

    res = run_bass_kernel_spmd(nc, in_maps, core_ids=list(range(N_CORES)),
                               **spmd_kwargs)
    # per-core out: [L*D, COLS_PER_CORE] transposed; gather + re-transpose
    outT = np.concatenate(
        [np.asarray(r["out"], np.float32) for r in res.results], axis=1)
    out = np.ascontiguousarray(outT.T)                              # [B, L*D]
    return out, res


def kernel(x, W, b):
    out, _ = run_shards(x, W, b)
    return out
